# revision 6
# baseline (speedup 1.0000x reference)
"""DecoderRNN single-step (embed+ReLU -> GRU cell -> vocab projection -> log_softmax)
distributed over 8 trn2 NeuronCores.

Sharding: vocab dimension of the output projection is sharded 8 ways
(50257 padded to 51200 -> 6400 rows/core). The GRU hidden dim is also
sharded 8 ways (128 h-entries/core) with an AllGather of h_new; the
log-softmax normalizer uses an AllGather of per-core partial sum-exp.
The embedding gather is resolved host-side (the owning shard's row is
staged to each core); ReLU happens on device.

Engine plan per core: the 26 MB W_out shard streams via DMA in 5 chunks;
DVE does the elementwise mults (stride-0-repeat second operand) and part
of the free-axis reductions (chunked tensor_reduce), ACT does the rest of
the reductions (Copy+accum_out) plus exp/ln.  All ACT functions stay in
the `natural_log_exp_and_others` table set (sigmoid/tanh are computed via
exp + DVE reciprocal) so only one ACT table load happens.
"""
import numpy as np

H = 1024
V = 50257
NCORES = 8
VP = 51200           # V padded to 8*128*50
VS = VP // NCORES    # 6400 rows per core
TILES = VS // 128    # 50 vocab tiles of 128 per core
CHUNK = 5            # tiles per DMA chunk
NCHUNKS = TILES // CHUNK
DVE_T = 2            # per chunk: tiles reduced on DVE (rest on ACT)
GP_CHUNKS = (7, 8, 9)  # chunks whose elementwise mult runs on GpSimd
GP_DVE_T = 4         # DVE reduce share for GpSimd-multed chunks
PAD_BIAS = -10000.0  # bias for padded vocab rows: exp() underflows to exactly 0
BF16_W = True        # stream W_out / GRU weights as bf16 (f32 accumulate)

_CACHE: dict = {}


def _bcast128(bass, ap_1d):
    """AP that reads a 1-D DRAM row replicated across all 128 partitions."""
    return bass.AP(tensor=ap_1d.tensor, offset=ap_1d.offset, ap=[[0, 128], *ap_1d.ap])


def _rep(bass, sb2d, k):
    """[128, N] SBUF tile viewed as [128, k, N] with the free dim repeated."""
    return bass.AP(tensor=sb2d.tensor, offset=sb2d.offset,
                   ap=[sb2d.ap[0], [0, k], sb2d.ap[1]])


def _build_nc(repeat=1):
    from concourse import bass, tile, mybir, bacc

    dt = mybir.dt
    AF = mybir.ActivationFunctionType
    wdt = dt.bfloat16 if BF16_W else dt.float32

    nc = bacc.Bacc("TRN2", target_bir_lowering=False, debug=False, num_devices=NCORES)

    wq = nc.dram_tensor("wq", [VS, H], wdt, kind="ExternalInput")
    wih3 = nc.dram_tensor("wih3", [128, 3, H], wdt, kind="ExternalInput")
    whh3 = nc.dram_tensor("whh3", [128, 3, H], wdt, kind="ExternalInput")
    bih3 = nc.dram_tensor("bih3", [128, 3], dt.float32, kind="ExternalInput")
    bhh3 = nc.dram_tensor("bhh3", [128, 3], dt.float32, kind="ExternalInput")
    bq = nc.dram_tensor("bq", [128, TILES], dt.float32, kind="ExternalInput")
    embrow = nc.dram_tensor("embrow", [H], dt.float32, kind="ExternalInput")
    hrow = nc.dram_tensor("hrow", [H], dt.float32, kind="ExternalInput")
    hslice = nc.dram_tensor("hslice", [128, 1], dt.float32, kind="ExternalInput")

    lp_out = nc.dram_tensor("lp_out", [128, TILES], dt.float32, kind="ExternalOutput")
    hnew_out = nc.dram_tensor("hnew_out", [NCORES * 128, 1], dt.float32,
                              kind="ExternalOutput")

    RG = [list(range(NCORES))]

    with tile.TileContext(nc) as tc:
        with (
            tc.tile_pool(name="singles", bufs=1) as singles,
            tc.tile_pool(name="wqp", bufs=6 if BF16_W else 3) as wqp,
            tc.tile_pool(name="scratch", bufs=2) as scratch,
            tc.tile_pool(name="dram", bufs=1, space="DRAM") as dram,
        ):
          for rep in range(repeat):
            # ---- stage GRU weights + small tensors first (they head the serial
            # chain: gates -> h_new -> AllGather -> projection) ----
            wih_sb = singles.tile([128, 3, H], wdt)
            nc.sync.dma_start(out=wih_sb, in_=wih3[:])
            whh_sb = singles.tile([128, 3, H], wdt)
            nc.sync.dma_start(out=whh_sb, in_=whh3[:])
            bih_sb = singles.tile([128, 3], dt.float32)
            nc.sync.dma_start(out=bih_sb, in_=bih3[:])
            bhh_sb = singles.tile([128, 3], dt.float32)
            nc.sync.dma_start(out=bhh_sb, in_=bhh3[:])
            bq_sb = singles.tile([128, TILES], dt.float32)
            nc.sync.dma_start(out=bq_sb, in_=bq[:])
            hs_sb = singles.tile([128, 1], dt.float32)
            nc.sync.dma_start(out=hs_sb, in_=hslice[:])

            xb = singles.tile([128, H], dt.float32)
            nc.gpsimd.dma_start(out=xb, in_=_bcast128(bass, embrow[:]))
            hb = singles.tile([128, H], dt.float32)
            nc.gpsimd.dma_start(out=hb, in_=_bcast128(bass, hrow[:]))

            xr = singles.tile([128, H], dt.float32)
            nc.scalar.activation(out=xr, in_=xb, func=AF.Relu)

            # ---- GRU cell, h-sharded: this core computes h_new[c*128:(c+1)*128].
            # One mult + one chunked reduce per weight matrix (3 gates at once).
            pih = scratch.tile([128, 3, H], dt.float32, tag="gprod")
            nc.vector.tensor_mul(pih, wih_sb, _rep(bass, xr, 3))
            gi3 = singles.tile([128, 3], dt.float32)
            nc.vector.tensor_reduce(out=gi3, in_=pih, axis=mybir.AxisListType.X,
                                    op=mybir.AluOpType.add)
            phh = scratch.tile([128, 3, H], dt.float32, tag="gprod")
            nc.vector.tensor_mul(phh, whh_sb, _rep(bass, hb, 3))
            gh3 = singles.tile([128, 3], dt.float32)
            nc.vector.tensor_reduce(out=gh3, in_=phh, axis=mybir.AxisListType.X,
                                    op=mybir.AluOpType.add)

            # negated bias sums for the exp-form gates
            bsum = singles.tile([128, 3], dt.float32)
            nc.vector.tensor_add(bsum, bih_sb, bhh_sb)
            nbsum = singles.tile([128, 3], dt.float32)
            nc.vector.tensor_scalar_mul(nbsum, bsum, -1.0)

            # r = sigmoid(gi_r + gh_r + b_r) = 1/(1+exp(-(pre + b_r)))
            one_t = singles.tile([128, 1], dt.float32)
            nc.vector.memset(one_t, 1.0)
            gates = {}
            for gidx, gname in ((0, "r"), (1, "z")):
                pre = singles.tile([128, 1], dt.float32, name=f"pre_{gname}")
                nc.vector.tensor_add(pre, gi3[:, gidx:gidx + 1], gh3[:, gidx:gidx + 1])
                eneg = singles.tile([128, 1], dt.float32, name=f"eneg_{gname}")
                nc.scalar.activation(out=eneg, in_=pre, func=AF.Exp,
                                     scale=-1.0, bias=nbsum[:, gidx:gidx + 1])
                ep1 = singles.tile([128, 1], dt.float32, name=f"ep1_{gname}")
                nc.vector.tensor_add(ep1, eneg, one_t)
                sig = singles.tile([128, 1], dt.float32, name=f"sig_{gname}")
                nc.vector.reciprocal(sig, ep1)
                gates[gname] = sig

            # n = tanh(y), y = gi_n + b_ih_n + r*(gh_n + b_hh_n)
            u_g = singles.tile([128, 1], dt.float32)
            nc.vector.tensor_scalar(u_g, gh3[:, 2:3], bhh_sb[:, 2:3], None,
                                    op0=mybir.AluOpType.add)
            v_g = singles.tile([128, 1], dt.float32)
            nc.vector.tensor_mul(v_g, gates["r"], u_g)
            w_g = singles.tile([128, 1], dt.float32)
            nc.vector.tensor_add(w_g, gi3[:, 2:3], v_g)
            # tanh(y) = 2/(1+exp(-2y)) - 1 ; exp arg = -2y - 2*b_ih_n
            nb2 = singles.tile([128, 1], dt.float32)
            nc.vector.tensor_scalar_mul(nb2, bih_sb[:, 2:3], -2.0)
            en = singles.tile([128, 1], dt.float32)
            nc.scalar.activation(out=en, in_=w_g, func=AF.Exp, scale=-2.0, bias=nb2)
            en1 = singles.tile([128, 1], dt.float32)
            nc.vector.tensor_add(en1, en, one_t)
            rec_n = singles.tile([128, 1], dt.float32)
            nc.vector.reciprocal(rec_n, en1)
            n_g = singles.tile([128, 1], dt.float32)
            nc.vector.tensor_scalar(n_g, rec_n, 2.0, -1.0,
                                    op0=mybir.AluOpType.mult, op1=mybir.AluOpType.add)

            # h_new = n + z*(h - n)
            d_g = singles.tile([128, 1], dt.float32)
            nc.vector.tensor_sub(d_g, hs_sb, n_g)
            e_g = singles.tile([128, 1], dt.float32)
            nc.vector.tensor_mul(e_g, gates["z"], d_g)
            hn_g = singles.tile([128, 1], dt.float32)
            nc.vector.tensor_add(hn_g, n_g, e_g)

            # ---- AllGather h_new across the 8 cores, broadcast to 128 partitions
            hag_in = dram.tile([128, 1], dt.float32)
            nc.sync.dma_start(out=hag_in[:], in_=hn_g)
            hag_out = dram.tile([NCORES * 128, 1], dt.float32, addr_space="Shared")
            nc.gpsimd.collective_compute(
                "AllGather", mybir.AluOpType.bypass, replica_groups=RG,
                ins=[hag_in.opt()], outs=[hag_out.opt()],
            )
            nc.sync.dma_start(out=hnew_out[:], in_=hag_out[:])
            hnb_bf = singles.tile([128, H], dt.bfloat16)
            hag_flat = hag_out[:, 0]
            nc.gpsimd.dma_start(out=hnb_bf, in_=_bcast128(bass, hag_flat))
            hnb_rep = singles.tile([128, CHUNK * H], dt.bfloat16)
            nc.vector.tensor_copy(hnb_rep, _rep(bass, hnb_bf, CHUNK))

            # ---- vocab projection: stream W_out shard; logits col tau gets
            # dot(W[v,:], h_new).  Reduces split between DVE and ACT.
            lg_sb = singles.tile([128, TILES], dt.float32)
            for j in range(NCHUNKS):
                wqc = wqp.tile([128, CHUNK * H], wdt, name=f"wqc_{j}", tag="wqc")
                src = wq[j * CHUNK * 128:(j + 1) * CHUNK * 128, :].rearrange(
                    "(s p) h -> p s h", p=128)
                wqc3 = bass.AP(tensor=wqc.tensor, offset=wqc.offset,
                               ap=[wqc.ap[0], [H, CHUNK], [1, H]])
                nc.sync.dma_start(out=wqc3, in_=src)
                prodc = scratch.tile([128, CHUNK * H], dt.bfloat16, tag="prodc",
                                     name=f"prodc_{j}")
                dve_t = GP_DVE_T if j in GP_CHUNKS else DVE_T
                if j in GP_CHUNKS:
                    nc.gpsimd.tensor_mul(prodc, wqc, hnb_rep)
                else:
                    nc.vector.tensor_mul(prodc, wqc, hnb_rep)
                t0 = j * CHUNK
                prodc3 = bass.AP(tensor=prodc.tensor, offset=prodc.offset,
                                 ap=[prodc.ap[0], [H, CHUNK], [1, H]])
                # DVE: one chunked reduce for the first dve_t tiles
                nc.vector.tensor_reduce(
                    out=lg_sb[:, t0:t0 + dve_t], in_=prodc3[:, 0:dve_t, :],
                    axis=mybir.AxisListType.X, op=mybir.AluOpType.add)
                # ACT: per-tile Copy+accum for the rest
                for s in range(dve_t, CHUNK):
                    tau = t0 + s
                    junkq = scratch.tile([128, H], dt.float32, tag="junk",
                                         name=f"junkq_{tau}")
                    nc.scalar.activation(out=junkq, in_=prodc3[:, s, :], func=AF.Copy,
                                         accum_out=lg_sb[:, tau:tau + 1])

            # ---- log-softmax (no max shift: logits ~ N(0,1), fp32-safe)
            lgb = singles.tile([128, TILES], dt.float32)
            nc.vector.tensor_add(lgb, lg_sb, bq_sb)
            e_scr = singles.tile([128, TILES], dt.float32)
            es = singles.tile([128, 1], dt.float32)
            nc.scalar.activation(out=e_scr, in_=lgb, func=AF.Exp, accum_out=es)

            es_in = dram.tile([128, 1], dt.float32)
            nc.sync.dma_start(out=es_in[:], in_=es)
            es_out = dram.tile([NCORES * 128, 1], dt.float32, addr_space="Shared")
            nc.gpsimd.collective_compute(
                "AllGather", mybir.AluOpType.bypass, replica_groups=RG,
                ins=[es_in.opt()], outs=[es_out.opt()],
            )
            esb = singles.tile([128, NCORES * 128], dt.float32)
            es_flat = es_out[:, 0]
            nc.gpsimd.dma_start(out=esb, in_=_bcast128(bass, es_flat))
            s_tot = singles.tile([128, 1], dt.float32)
            nc.vector.tensor_reduce(out=s_tot, in_=esb, axis=mybir.AxisListType.X,
                                    op=mybir.AluOpType.add)
            logz = singles.tile([128, 1], dt.float32)
            nc.scalar.activation(out=logz, in_=s_tot, func=AF.Ln)

            lp_sb = singles.tile([128, TILES], dt.float32)
            nc.vector.tensor_scalar_sub(lp_sb, lgb, logz)
            nc.sync.dma_start(out=lp_out[:], in_=lp_sb)

    nc.compile()
    return nc


def _prep_in_maps(input, hidden, emb, W_ih, W_hh, b_ih, b_hh, W_out, b_out):
    import ml_dtypes

    f32 = np.float32
    wdt = ml_dtypes.bfloat16 if BF16_W else np.float32
    idx = int(np.asarray(input).reshape(-1)[0])
    emb_row = np.ascontiguousarray(np.asarray(emb, dtype=f32)[idx])        # [H]
    h_row = np.ascontiguousarray(np.asarray(hidden, dtype=f32).reshape(H))  # [H]
    W_ih = np.asarray(W_ih, dtype=f32)
    W_hh = np.asarray(W_hh, dtype=f32)
    b_ih = np.asarray(b_ih, dtype=f32)
    b_hh = np.asarray(b_hh, dtype=f32)
    W_out = np.asarray(W_out, dtype=f32)
    b_out = np.asarray(b_out, dtype=f32)

    wih_g = W_ih.reshape(3, H, H)   # [gate, out, in]
    whh_g = W_hh.reshape(3, H, H)
    bih_g = b_ih.reshape(3, H)
    bhh_g = b_hh.reshape(3, H)

    in_maps = []
    for c in range(NCORES):
        lo, hi = c * VS, (c + 1) * VS
        if hi <= V:
            wq_c = W_out[lo:hi]
            b_c = b_out[lo:hi]
        else:
            n_real = max(0, V - lo)
            wq_c = np.zeros((VS, H), dtype=f32)
            wq_c[:n_real] = W_out[lo:V]
            b_c = np.full(VS, PAD_BIAS, dtype=f32)
            b_c[:n_real] = b_out[lo:V]
        sl = slice(c * 128, (c + 1) * 128)
        in_maps.append({
            "wq": np.ascontiguousarray(wq_c.astype(wdt)),
            "wih3": np.ascontiguousarray(
                wih_g[:, sl, :].transpose(1, 0, 2).astype(wdt)),
            "whh3": np.ascontiguousarray(
                whh_g[:, sl, :].transpose(1, 0, 2).astype(wdt)),
            "bih3": np.ascontiguousarray(bih_g[:, sl].T),
            "bhh3": np.ascontiguousarray(bhh_g[:, sl].T),
            "bq": np.ascontiguousarray(b_c.reshape(TILES, 128).T),
            "embrow": emb_row,
            "hrow": h_row,
            "hslice": np.ascontiguousarray(h_row[sl, None]),
        })
    return in_maps


def kernel(input, hidden, emb, W_ih, W_hh, b_ih, b_hh, W_out, b_out, _trace=False):
    from concourse.bass_utils import run_bass_kernel_spmd

    if "nc" not in _CACHE:
        _CACHE["nc"] = _build_nc()
    nc = _CACHE["nc"]

    in_maps = _prep_in_maps(input, hidden, emb, W_ih, W_hh, b_ih, b_hh, W_out, b_out)
    res = run_bass_kernel_spmd(nc, in_maps, core_ids=list(range(NCORES)),
                               trace=_trace)
    parts = []
    for c in range(NCORES):
        lp_c = res.results[c]["lp_out"]          # [128, TILES], vocab v = t*128+p
        parts.append(np.ascontiguousarray(lp_c.T).reshape(VS))
    logprobs = np.concatenate(parts)[:V].reshape(1, V).astype(np.float32)
    h_new = res.results[0]["hnew_out"].reshape(1, 1, H).astype(np.float32)
    if _trace:
        kernel._last_results = res
    return logprobs, h_new


# revision 7
# speedup vs baseline: 1.4081x; 1.4081x over previous
"""DecoderRNN single-step (embed+ReLU -> GRU cell -> vocab projection -> log_softmax)
distributed over 8 trn2 NeuronCores.

Sharding: vocab dimension of the output projection is sharded 8 ways
(50257 padded to 51200 -> 6400 rows/core). The GRU hidden dim is also
sharded 8 ways (128 h-entries/core) with an AllGather of h_new; the
log-softmax normalizer uses an AllGather of per-core partial sum-exp.
The embedding gather is resolved host-side (the owning shard's row is
staged to each core); ReLU happens on device.

Engine plan per core: the 26 MB W_out shard streams via DMA in 5 chunks;
DVE does the elementwise mults (stride-0-repeat second operand) and part
of the free-axis reductions (chunked tensor_reduce), ACT does the rest of
the reductions (Copy+accum_out) plus exp/ln.  All ACT functions stay in
the `natural_log_exp_and_others` table set (sigmoid/tanh are computed via
exp + DVE reciprocal) so only one ACT table load happens.
"""
import numpy as np

H = 1024
V = 50257
NCORES = 8
VP = 51200           # V padded to 8*128*50
VS = VP // NCORES    # 6400 rows per core
TILES = VS // 128    # 50 vocab tiles of 128 per core
CHUNK = 5            # tiles per DMA chunk
NCHUNKS = TILES // CHUNK
DVE_T = 2            # per chunk: tiles reduced on DVE (rest on ACT)
GP_CHUNKS = ()       # chunks whose elementwise mult runs on GpSimd (empirically a loss)
GP_DVE_T = 4         # DVE reduce share for GpSimd-multed chunks
PAD_BIAS = -10000.0  # bias for padded vocab rows: exp() underflows to exactly 0
BF16_W = True        # stream W_out / GRU weights as bf16 (f32 accumulate)

_CACHE: dict = {}


def _bcast128(bass, ap_1d):
    """AP that reads a 1-D DRAM row replicated across all 128 partitions."""
    return bass.AP(tensor=ap_1d.tensor, offset=ap_1d.offset, ap=[[0, 128], *ap_1d.ap])


def _rep(bass, sb2d, k):
    """[128, N] SBUF tile viewed as [128, k, N] with the free dim repeated."""
    return bass.AP(tensor=sb2d.tensor, offset=sb2d.offset,
                   ap=[sb2d.ap[0], [0, k], sb2d.ap[1]])


def _build_nc(repeat=1):
    from concourse import bass, tile, mybir, bacc

    dt = mybir.dt
    AF = mybir.ActivationFunctionType
    wdt = dt.bfloat16 if BF16_W else dt.float32

    nc = bacc.Bacc("TRN2", target_bir_lowering=False, debug=False, num_devices=NCORES)

    wq = nc.dram_tensor("wq", [VS, H], wdt, kind="ExternalInput")
    wih3 = nc.dram_tensor("wih3", [128, 3, H], wdt, kind="ExternalInput")
    whh3 = nc.dram_tensor("whh3", [128, 3, H], wdt, kind="ExternalInput")
    bih3 = nc.dram_tensor("bih3", [128, 3], dt.float32, kind="ExternalInput")
    bhh3 = nc.dram_tensor("bhh3", [128, 3], dt.float32, kind="ExternalInput")
    bq = nc.dram_tensor("bq", [128, TILES], dt.float32, kind="ExternalInput")
    embrow = nc.dram_tensor("embrow", [H], dt.float32, kind="ExternalInput")
    hrow = nc.dram_tensor("hrow", [H], dt.float32, kind="ExternalInput")
    hslice = nc.dram_tensor("hslice", [128, 1], dt.float32, kind="ExternalInput")

    lp_out = nc.dram_tensor("lp_out", [128, TILES], dt.float32, kind="ExternalOutput")
    hnew_out = nc.dram_tensor("hnew_out", [NCORES * 128, 1], dt.float32,
                              kind="ExternalOutput")

    RG = [list(range(NCORES))]

    with tile.TileContext(nc) as tc:
        with (
            tc.tile_pool(name="singles", bufs=1) as singles,
            tc.tile_pool(name="wqp", bufs=6 if BF16_W else 3) as wqp,
            tc.tile_pool(name="scratch", bufs=2) as scratch,
            tc.tile_pool(name="dram", bufs=1, space="DRAM") as dram,
        ):
          for rep in range(repeat):
            # ---- stage GRU weights + small tensors first (they head the serial
            # chain: gates -> h_new -> AllGather -> projection) ----
            wih_sb = singles.tile([128, 3, H], wdt)
            nc.sync.dma_start(out=wih_sb, in_=wih3[:])
            whh_sb = singles.tile([128, 3, H], wdt)
            nc.sync.dma_start(out=whh_sb, in_=whh3[:])
            bih_sb = singles.tile([128, 3], dt.float32)
            nc.sync.dma_start(out=bih_sb, in_=bih3[:])
            bhh_sb = singles.tile([128, 3], dt.float32)
            nc.sync.dma_start(out=bhh_sb, in_=bhh3[:])
            bq_sb = singles.tile([128, TILES], dt.float32)
            nc.sync.dma_start(out=bq_sb, in_=bq[:])
            hs_sb = singles.tile([128, 1], dt.float32)
            nc.sync.dma_start(out=hs_sb, in_=hslice[:])

            xb = singles.tile([128, H], dt.float32)
            nc.gpsimd.dma_start(out=xb, in_=_bcast128(bass, embrow[:]))
            hb = singles.tile([128, H], dt.float32)
            nc.gpsimd.dma_start(out=hb, in_=_bcast128(bass, hrow[:]))

            xr = singles.tile([128, H], dt.float32)
            nc.scalar.activation(out=xr, in_=xb, func=AF.Relu)

            # ---- GRU cell, h-sharded: this core computes h_new[c*128:(c+1)*128].
            # One mult + one chunked reduce per weight matrix (3 gates at once).
            pih = scratch.tile([128, 3, H], dt.float32, tag="gprod")
            nc.vector.tensor_mul(pih, wih_sb, _rep(bass, xr, 3))
            gi3 = singles.tile([128, 3], dt.float32)
            nc.vector.tensor_reduce(out=gi3, in_=pih, axis=mybir.AxisListType.X,
                                    op=mybir.AluOpType.add)
            phh = scratch.tile([128, 3, H], dt.float32, tag="gprod")
            nc.vector.tensor_mul(phh, whh_sb, _rep(bass, hb, 3))
            gh3 = singles.tile([128, 3], dt.float32)
            nc.vector.tensor_reduce(out=gh3, in_=phh, axis=mybir.AxisListType.X,
                                    op=mybir.AluOpType.add)

            # negated bias sums for the exp-form gates
            bsum = singles.tile([128, 3], dt.float32)
            nc.vector.tensor_add(bsum, bih_sb, bhh_sb)
            nbsum = singles.tile([128, 3], dt.float32)
            nc.vector.tensor_scalar_mul(nbsum, bsum, -1.0)

            # r = sigmoid(gi_r + gh_r + b_r) = 1/(1+exp(-(pre + b_r)))
            one_t = singles.tile([128, 1], dt.float32)
            nc.vector.memset(one_t, 1.0)
            gates = {}
            for gidx, gname in ((0, "r"), (1, "z")):
                pre = singles.tile([128, 1], dt.float32, name=f"pre_{gname}")
                nc.vector.tensor_add(pre, gi3[:, gidx:gidx + 1], gh3[:, gidx:gidx + 1])
                eneg = singles.tile([128, 1], dt.float32, name=f"eneg_{gname}")
                nc.scalar.activation(out=eneg, in_=pre, func=AF.Exp,
                                     scale=-1.0, bias=nbsum[:, gidx:gidx + 1])
                ep1 = singles.tile([128, 1], dt.float32, name=f"ep1_{gname}")
                nc.vector.tensor_add(ep1, eneg, one_t)
                sig = singles.tile([128, 1], dt.float32, name=f"sig_{gname}")
                nc.vector.reciprocal(sig, ep1)
                gates[gname] = sig

            # n = tanh(y), y = gi_n + b_ih_n + r*(gh_n + b_hh_n)
            u_g = singles.tile([128, 1], dt.float32)
            nc.vector.tensor_scalar(u_g, gh3[:, 2:3], bhh_sb[:, 2:3], None,
                                    op0=mybir.AluOpType.add)
            v_g = singles.tile([128, 1], dt.float32)
            nc.vector.tensor_mul(v_g, gates["r"], u_g)
            w_g = singles.tile([128, 1], dt.float32)
            nc.vector.tensor_add(w_g, gi3[:, 2:3], v_g)
            # tanh(y) = 2/(1+exp(-2y)) - 1 ; exp arg = -2y - 2*b_ih_n
            nb2 = singles.tile([128, 1], dt.float32)
            nc.vector.tensor_scalar_mul(nb2, bih_sb[:, 2:3], -2.0)
            en = singles.tile([128, 1], dt.float32)
            nc.scalar.activation(out=en, in_=w_g, func=AF.Exp, scale=-2.0, bias=nb2)
            en1 = singles.tile([128, 1], dt.float32)
            nc.vector.tensor_add(en1, en, one_t)
            rec_n = singles.tile([128, 1], dt.float32)
            nc.vector.reciprocal(rec_n, en1)
            n_g = singles.tile([128, 1], dt.float32)
            nc.vector.tensor_scalar(n_g, rec_n, 2.0, -1.0,
                                    op0=mybir.AluOpType.mult, op1=mybir.AluOpType.add)

            # h_new = n + z*(h - n)
            d_g = singles.tile([128, 1], dt.float32)
            nc.vector.tensor_sub(d_g, hs_sb, n_g)
            e_g = singles.tile([128, 1], dt.float32)
            nc.vector.tensor_mul(e_g, gates["z"], d_g)
            hn_g = singles.tile([128, 1], dt.float32)
            nc.vector.tensor_add(hn_g, n_g, e_g)

            # ---- AllGather h_new across the 8 cores, broadcast to 128 partitions
            hag_in = dram.tile([128, 1], dt.float32)
            nc.sync.dma_start(out=hag_in[:], in_=hn_g)
            hag_out = dram.tile([NCORES * 128, 1], dt.float32, addr_space="Shared")
            nc.gpsimd.collective_compute(
                "AllGather", mybir.AluOpType.bypass, replica_groups=RG,
                ins=[hag_in.opt()], outs=[hag_out.opt()],
            )
            nc.sync.dma_start(out=hnew_out[:], in_=hag_out[:])
            hnb_bf = singles.tile([128, H], dt.bfloat16)
            hag_flat = hag_out[:, 0]
            nc.gpsimd.dma_start(out=hnb_bf, in_=_bcast128(bass, hag_flat))
            hnb_rep = singles.tile([128, CHUNK * H], dt.bfloat16)
            nc.vector.tensor_copy(hnb_rep, _rep(bass, hnb_bf, CHUNK))

            # ---- vocab projection: stream W_out shard; logits col tau gets
            # dot(W[v,:], h_new).  Reduces split between DVE and ACT.
            lg_sb = singles.tile([128, TILES], dt.float32)
            for j in range(NCHUNKS):
                wqc = wqp.tile([128, CHUNK * H], wdt, name=f"wqc_{j}", tag="wqc")
                src = wq[j * CHUNK * 128:(j + 1) * CHUNK * 128, :].rearrange(
                    "(s p) h -> p s h", p=128)
                wqc3 = bass.AP(tensor=wqc.tensor, offset=wqc.offset,
                               ap=[wqc.ap[0], [H, CHUNK], [1, H]])
                nc.sync.dma_start(out=wqc3, in_=src)
                prodc = scratch.tile([128, CHUNK * H], dt.bfloat16, tag="prodc",
                                     name=f"prodc_{j}")
                dve_t = GP_DVE_T if j in GP_CHUNKS else DVE_T
                if j in GP_CHUNKS:
                    nc.gpsimd.tensor_mul(prodc, wqc, hnb_rep)
                else:
                    nc.vector.tensor_mul(prodc, wqc, hnb_rep)
                t0 = j * CHUNK
                prodc3 = bass.AP(tensor=prodc.tensor, offset=prodc.offset,
                                 ap=[prodc.ap[0], [H, CHUNK], [1, H]])
                # DVE: one chunked reduce for the first dve_t tiles
                nc.vector.tensor_reduce(
                    out=lg_sb[:, t0:t0 + dve_t], in_=prodc3[:, 0:dve_t, :],
                    axis=mybir.AxisListType.X, op=mybir.AluOpType.add)
                # ACT: per-tile Copy+accum for the rest
                for s in range(dve_t, CHUNK):
                    tau = t0 + s
                    junkq = scratch.tile([128, H], dt.float32, tag="junk",
                                         name=f"junkq_{tau}")
                    nc.scalar.activation(out=junkq, in_=prodc3[:, s, :], func=AF.Copy,
                                         accum_out=lg_sb[:, tau:tau + 1])

            # ---- log-softmax (no max shift: logits ~ N(0,1), fp32-safe)
            lgb = singles.tile([128, TILES], dt.float32)
            nc.vector.tensor_add(lgb, lg_sb, bq_sb)
            e_scr = singles.tile([128, TILES], dt.float32)
            es = singles.tile([128, 1], dt.float32)
            nc.scalar.activation(out=e_scr, in_=lgb, func=AF.Exp, accum_out=es)

            es_in = dram.tile([128, 1], dt.float32)
            nc.sync.dma_start(out=es_in[:], in_=es)
            es_out = dram.tile([NCORES * 128, 1], dt.float32, addr_space="Shared")
            nc.gpsimd.collective_compute(
                "AllGather", mybir.AluOpType.bypass, replica_groups=RG,
                ins=[es_in.opt()], outs=[es_out.opt()],
            )
            esb = singles.tile([128, NCORES * 128], dt.float32)
            es_flat = es_out[:, 0]
            nc.gpsimd.dma_start(out=esb, in_=_bcast128(bass, es_flat))
            s_tot = singles.tile([128, 1], dt.float32)
            nc.vector.tensor_reduce(out=s_tot, in_=esb, axis=mybir.AxisListType.X,
                                    op=mybir.AluOpType.add)
            logz = singles.tile([128, 1], dt.float32)
            nc.scalar.activation(out=logz, in_=s_tot, func=AF.Ln)

            lp_sb = singles.tile([128, TILES], dt.float32)
            nc.vector.tensor_scalar_sub(lp_sb, lgb, logz)
            nc.sync.dma_start(out=lp_out[:], in_=lp_sb)

    nc.compile()
    return nc


def _prep_in_maps(input, hidden, emb, W_ih, W_hh, b_ih, b_hh, W_out, b_out):
    import ml_dtypes

    f32 = np.float32
    wdt = ml_dtypes.bfloat16 if BF16_W else np.float32
    idx = int(np.asarray(input).reshape(-1)[0])
    emb_row = np.ascontiguousarray(np.asarray(emb, dtype=f32)[idx])        # [H]
    h_row = np.ascontiguousarray(np.asarray(hidden, dtype=f32).reshape(H))  # [H]
    W_ih = np.asarray(W_ih, dtype=f32)
    W_hh = np.asarray(W_hh, dtype=f32)
    b_ih = np.asarray(b_ih, dtype=f32)
    b_hh = np.asarray(b_hh, dtype=f32)
    W_out = np.asarray(W_out, dtype=f32)
    b_out = np.asarray(b_out, dtype=f32)

    wih_g = W_ih.reshape(3, H, H)   # [gate, out, in]
    whh_g = W_hh.reshape(3, H, H)
    bih_g = b_ih.reshape(3, H)
    bhh_g = b_hh.reshape(3, H)

    in_maps = []
    for c in range(NCORES):
        lo, hi = c * VS, (c + 1) * VS
        if hi <= V:
            wq_c = W_out[lo:hi]
            b_c = b_out[lo:hi]
        else:
            n_real = max(0, V - lo)
            wq_c = np.zeros((VS, H), dtype=f32)
            wq_c[:n_real] = W_out[lo:V]
            b_c = np.full(VS, PAD_BIAS, dtype=f32)
            b_c[:n_real] = b_out[lo:V]
        sl = slice(c * 128, (c + 1) * 128)
        in_maps.append({
            "wq": np.ascontiguousarray(wq_c.astype(wdt)),
            "wih3": np.ascontiguousarray(
                wih_g[:, sl, :].transpose(1, 0, 2).astype(wdt)),
            "whh3": np.ascontiguousarray(
                whh_g[:, sl, :].transpose(1, 0, 2).astype(wdt)),
            "bih3": np.ascontiguousarray(bih_g[:, sl].T),
            "bhh3": np.ascontiguousarray(bhh_g[:, sl].T),
            "bq": np.ascontiguousarray(b_c.reshape(TILES, 128).T),
            "embrow": emb_row,
            "hrow": h_row,
            "hslice": np.ascontiguousarray(h_row[sl, None]),
        })
    return in_maps


def kernel(input, hidden, emb, W_ih, W_hh, b_ih, b_hh, W_out, b_out, _trace=False):
    from concourse.bass_utils import run_bass_kernel_spmd

    if "nc" not in _CACHE:
        _CACHE["nc"] = _build_nc()
    nc = _CACHE["nc"]

    in_maps = _prep_in_maps(input, hidden, emb, W_ih, W_hh, b_ih, b_hh, W_out, b_out)
    res = run_bass_kernel_spmd(nc, in_maps, core_ids=list(range(NCORES)),
                               trace=_trace)
    parts = []
    for c in range(NCORES):
        lp_c = res.results[c]["lp_out"]          # [128, TILES], vocab v = t*128+p
        parts.append(np.ascontiguousarray(lp_c.T).reshape(VS))
    logprobs = np.concatenate(parts)[:V].reshape(1, V).astype(np.float32)
    h_new = res.results[0]["hnew_out"].reshape(1, 1, H).astype(np.float32)
    if _trace:
        kernel._last_results = res
    return logprobs, h_new
